# revision 1
# baseline (speedup 1.0000x reference)
"""MoE expert-gating kernel for 8 Trainium2 NeuronCores.

Problem (nn_ExpertGating): router MLP (H->H relu, H->E) + softmax + top-2
gating + weighted combine of per-expert outputs.

Sharding: data-parallel over the B*S=8192 tokens -> 1024 tokens per core.
Each core runs the full router for its tokens and combines its slice of all
8 experts' outputs.  No collectives needed; host concatenates the slices.

Per-core layout (T=1024 tokens, H=1024, E=8):
  x   [T, H]      hidden states slice
  eo  [E, T, H]   expert outputs slice
  w1  [H, H], b1 [H], w2 [H, E], b2 [E]  (replicated router weights)
  out [T, H]

Pipeline on each core:
  1. transpose x via PE (needed because matmul contracts over the partition
     dim): xT[h, t]
  2. hT = relu(W1.T @ xT + b1)        (PE + ACT)
  3. logits[t, e] = hT.T @ W2 + b2    (PE)
  4. softmax over E, top-2 via max8 + match_replace -> dense gates [t, E]
  5. out[t] = sum_e gates[t, e] * eo[e, t]   (DVE scalar_tensor_tensor)
"""

import numpy as np

B, S, H, E = 4, 2048, 1024, 8
N_CORES = 8
T = (B * S) // N_CORES  # tokens per core
P = 128  # partitions
TCH = T // P  # token chunks per core (8)
KT = H // P  # contraction tiles (8)

_compiled_nc = None


def _build():
    import concourse.bacc as bacc
    import concourse.bass as bass
    import concourse.tile as tile
    from concourse import mybir
    from concourse.masks import make_identity

    f32 = mybir.dt.float32
    nc = bacc.Bacc("TRN2", target_bir_lowering=False, debug=False,
                   num_devices=N_CORES)

    x = nc.dram_tensor("x", [T, H], f32, kind="ExternalInput").ap()
    eo = nc.dram_tensor("eo", [E, T, H], f32, kind="ExternalInput").ap()
    w1 = nc.dram_tensor("w1", [H, H], f32, kind="ExternalInput").ap()
    b1 = nc.dram_tensor("b1", [H], f32, kind="ExternalInput").ap()
    w2 = nc.dram_tensor("w2", [H, E], f32, kind="ExternalInput").ap()
    b2 = nc.dram_tensor("b2", [E], f32, kind="ExternalInput").ap()
    out = nc.dram_tensor("out", [T, H], f32, kind="ExternalOutput").ap()

    with tile.TileContext(nc) as tc:
        with (
            tc.tile_pool(name="singles", bufs=1) as singles,
            tc.tile_pool(name="xpool", bufs=3) as xpool,
            tc.tile_pool(name="eopool", bufs=2) as eopool,
            tc.tile_pool(name="accpool", bufs=3) as accpool,
            tc.tile_pool(name="smalls", bufs=24) as smalls,
            tc.tile_pool(name="psumT", bufs=3, space="PSUM") as psumT,
            tc.tile_pool(name="psum2", bufs=2, space="PSUM") as psum2,
            tc.tile_pool(name="psum3", bufs=2, space="PSUM") as psum3,
        ):
            # ---- constants / weights (SP HWDGE ring) ----
            w1_sb = singles.tile([P, KT, H], f32)  # w1_sb[p,k,m] = W1[k*128+p, m]
            nc.sync.dma_start(out=w1_sb[:], in_=w1.rearrange("(k p) m -> p k m", p=P))
            w2_sb = singles.tile([P, KT, E], f32)  # w2_sb[p,k,e] = W2[k*128+p, e]
            nc.sync.dma_start(out=w2_sb[:], in_=w2.rearrange("(k p) e -> p k e", p=P))
            b1_sb = singles.tile([P, KT], f32)  # b1_sb[p,m] = b1[m*128+p]
            nc.sync.dma_start(out=b1_sb[:], in_=b1.rearrange("(m p) -> p m", p=P))
            b2_sb = singles.tile([P, E], f32)  # broadcast of b2 over partitions
            b2_bc = bass.AP(tensor=b2.tensor, offset=b2.offset,
                            ap=[[0, P], b2.ap[0]])
            nc.sync.dma_start(out=b2_sb[:], in_=b2_bc)
            ident = singles.tile([P, P], f32)
            make_identity(nc, ident[:])

            xT = singles.tile([P, KT, T], f32)  # xT[p,k,t] = x[t, k*128+p]
            hT = singles.tile([P, KT, T], f32)  # hT[p,m,t] = relu(x@W1+b1)[t, m*128+p]
            gates = singles.tile([P, TCH, E], f32)

            # ---- stage 1: load x and transpose on PE ----
            for tch in range(TCH):
                x_tile = xpool.tile([P, H], f32)
                nc.sync.dma_start(out=x_tile[:], in_=x[tch * P:(tch + 1) * P, :])
                for k in range(KT):
                    pt = psumT.tile([P, P], f32)
                    nc.tensor.transpose(pt[:], x_tile[:, k * P:(k + 1) * P], ident[:])
                    nc.scalar.copy(out=xT[:, k, tch * P:(tch + 1) * P], in_=pt[:])

            # ---- stage 2: hT = relu(W1.T @ xT + b1) ----
            NH = 2  # halves of the t range, 512 each (PSUM free-dim limit)
            for m in range(KT):
                for n in range(NH):
                    ps = psum2.tile([P, T // NH], f32)
                    for k in range(KT):
                        nc.tensor.matmul(
                            ps[:],
                            lhsT=w1_sb[:, k, m * P:(m + 1) * P],
                            rhs=xT[:, k, n * (T // NH):(n + 1) * (T // NH)],
                            start=(k == 0), stop=(k == KT - 1),
                        )
                    nc.scalar.activation(
                        out=hT[:, m, n * (T // NH):(n + 1) * (T // NH)], in_=ps[:],
                        func=mybir.ActivationFunctionType.Relu,
                        bias=b1_sb[:, m:m + 1], scale=1.0,
                    )

            # ---- stage 3+4: logits, softmax, top-2 gates ----
            for tch in range(TCH):
                psl = psum3.tile([P, E], f32)
                for k in range(KT):
                    nc.tensor.matmul(
                        psl[:],
                        lhsT=hT[:, k, tch * P:(tch + 1) * P],
                        rhs=w2_sb[:, k, :],
                        start=(k == 0), stop=(k == KT - 1),
                    )
                logits = smalls.tile([P, E], f32)
                nc.vector.tensor_add(logits[:], psl[:], b2_sb[:])
                negmax = smalls.tile([P, 1], f32)
                nc.vector.reduce_max(negmax[:], logits[:],
                                     axis=mybir.AxisListType.X, negate=True)
                exps = smalls.tile([P, E], f32)
                nc.scalar.activation(exps[:], logits[:],
                                     func=mybir.ActivationFunctionType.Exp,
                                     bias=negmax[:], scale=1.0)
                ssum = smalls.tile([P, 1], f32)
                nc.vector.reduce_sum(ssum[:], exps[:], axis=mybir.AxisListType.X)
                rs = smalls.tile([P, 1], f32)
                nc.vector.reciprocal(rs[:], ssum[:])
                probs = smalls.tile([P, E], f32)
                nc.vector.tensor_scalar_mul(probs[:], exps[:], rs[:])
                # top-2: find the 8 sorted maxes, keep the top 2, zap them out
                # of probs, subtract -> only top-2 probs survive as gates.
                mx8 = smalls.tile([P, 8], f32)
                nc.vector.max(mx8[:], probs[:])
                nc.vector.memset(mx8[:, 2:], 0.0)
                zap = smalls.tile([P, E], f32)
                nc.vector.match_replace(out=zap[:], in_to_replace=mx8[:],
                                        in_values=probs[:], imm_value=0.0)
                nc.vector.tensor_sub(gates[:, tch, :], probs[:], zap[:])

            # ---- stage 5: weighted combine (Pool/SWDGE ring for eo+out) ----
            for tch in range(TCH):
                eo_t = eopool.tile([P, E, H], f32)
                nc.gpsimd.dma_start(
                    out=eo_t[:],
                    in_=eo[:, tch * P:(tch + 1) * P, :].rearrange("e p h -> p e h"),
                )
                acc = accpool.tile([P, H], f32)
                nc.vector.tensor_scalar_mul(acc[:], eo_t[:, 0, :],
                                            gates[:, tch, 0:1])
                for e in range(1, E):
                    nc.vector.scalar_tensor_tensor(
                        out=acc[:], in0=eo_t[:, e, :],
                        scalar=gates[:, tch, e:e + 1], in1=acc[:],
                        op0=mybir.AluOpType.mult, op1=mybir.AluOpType.add,
                    )
                nc.gpsimd.dma_start(out=out[tch * P:(tch + 1) * P, :], in_=acc[:])

    nc.compile()
    return nc


def _get_nc():
    global _compiled_nc
    if _compiled_nc is None:
        _compiled_nc = _build()
    return _compiled_nc


def kernel(hidden_states, expert_outputs, W1, b1, W2, b2, k=2):
    from concourse.bass_utils import run_bass_kernel_spmd

    hs = np.ascontiguousarray(np.asarray(hidden_states, dtype=np.float32)).reshape(B * S, H)
    eo = np.ascontiguousarray(np.asarray(expert_outputs, dtype=np.float32)).reshape(E, B * S, H)
    w1 = np.ascontiguousarray(np.asarray(W1, dtype=np.float32))
    b1v = np.ascontiguousarray(np.asarray(b1, dtype=np.float32))
    w2 = np.ascontiguousarray(np.asarray(W2, dtype=np.float32))
    b2v = np.ascontiguousarray(np.asarray(b2, dtype=np.float32))

    in_maps = []
    for c in range(N_CORES):
        sl = slice(c * T, (c + 1) * T)
        in_maps.append({
            "x": np.ascontiguousarray(hs[sl]),
            "eo": np.ascontiguousarray(eo[:, sl, :]),
            "w1": w1, "b1": b1v, "w2": w2, "b2": b2v,
        })

    nc = _get_nc()
    res = run_bass_kernel_spmd(nc, in_maps, core_ids=list(range(N_CORES)))
    full = np.concatenate([res.results[c]["out"] for c in range(N_CORES)], axis=0)
    return full.reshape(B, S, H)


# revision 4
# speedup vs baseline: 1.3969x; 1.3969x over previous
"""MoE expert-gating kernel for 8 Trainium2 NeuronCores.

Problem (nn_ExpertGating): router MLP (H->H relu, H->E) + softmax + top-2
gating + weighted combine of per-expert outputs.

Sharding: data-parallel over the B*S=8192 tokens -> 1024 tokens per core.
Each core runs the full router for its tokens and combines its slice of all
8 experts' outputs.  No collectives needed; host concatenates the slices.

Per-core pipeline (T=1024 tokens, H=1024, E=8), fp32 throughout (top-2
selection needs fp32 logits: min top2/3 margin on this data is ~5e-6):
  1. transpose x via PE: xT[h, t]
  2. hT = relu(W1.T @ xT + b1)          (PE fp32 2-pass + ACT)
  3. logitsT[e, t] = W2.T @ hT + b2     (PE, W2 stationary -> tiny LDW;
     b2 folded into the PSUM->SBUF copy where experts sit on partitions)
  4. transpose logits chunks back to [t, E], softmax over E, top-2 via
     max8 + match_replace -> dense gates [t, E]
  5. out[t] = sum_e gates[t, e] * eo[e, t]   (DVE scalar_tensor_tensor)

Stage 2-4 run per 512-token half so the first half's combines overlap the
second half's router matmuls.  eo loads stream on the gpsimd (SWDGE) ring
from t=0; x/W/out live on the SP (HWDGE) ring.
"""

import numpy as np

B, S, H, E = 4, 2048, 1024, 8
N_CORES = 8
T = (B * S) // N_CORES  # tokens per core
P = 128  # partitions
TCH = T // P  # token chunks per core (8)
KT = H // P  # contraction tiles (8)
NH = 2  # halves of the token range for stage 2/3
HAL = T // NH  # 512
EH = E // 2  # experts per eo tile (SBUF budget)

_compiled_nc = None


def _build():
    import concourse.bacc as bacc
    import concourse.bass as bass
    import concourse.tile as tile
    from concourse import mybir
    from concourse.masks import make_identity

    f32 = mybir.dt.float32
    nc = bacc.Bacc("TRN2", target_bir_lowering=False, debug=False,
                   num_devices=N_CORES)

    x = nc.dram_tensor("x", [T, H], f32, kind="ExternalInput").ap()
    eo = nc.dram_tensor("eo", [E, T, H], f32, kind="ExternalInput").ap()
    w1 = nc.dram_tensor("w1", [H, H], f32, kind="ExternalInput").ap()
    b1 = nc.dram_tensor("b1", [H], f32, kind="ExternalInput").ap()
    w2 = nc.dram_tensor("w2", [H, E], f32, kind="ExternalInput").ap()
    b2 = nc.dram_tensor("b2", [E], f32, kind="ExternalInput").ap()
    out = nc.dram_tensor("out", [T, H], f32, kind="ExternalOutput").ap()

    with tile.TileContext(nc) as tc:
        with (
            tc.tile_pool(name="singles", bufs=1) as singles,
            tc.tile_pool(name="xpool", bufs=1) as xpool,
            tc.tile_pool(name="eopool", bufs=3) as eopool,
            tc.tile_pool(name="accpool", bufs=3) as accpool,
            tc.tile_pool(name="smalls", bufs=8) as smalls,
            tc.tile_pool(name="ltpool", bufs=2) as ltpool,
            tc.tile_pool(name="psumT", bufs=2, space="PSUM") as psumT,
            tc.tile_pool(name="psum2", bufs=2, space="PSUM") as psum2,
            tc.tile_pool(name="psum3", bufs=2, space="PSUM") as psum3,
            tc.tile_pool(name="psumL", bufs=2, space="PSUM") as psumL,
        ):
            # ---- eo loads stream on the Pool/SWDGE ring from t=0 ----
            # two tiles per token chunk (experts 0-3 / 4-7), 2 MB per DMA
            eo_tiles = []
            for tch in range(TCH):
                pair = []
                for g in range(E // EH):
                    eo_t = eopool.tile([P, EH, H], f32, tag="eo")
                    nc.gpsimd.dma_start(
                        out=eo_t[:],
                        in_=eo[g * EH:(g + 1) * EH,
                               tch * P:(tch + 1) * P, :].rearrange(
                            "e p h -> p e h"),
                    )
                    pair.append(eo_t)
                eo_tiles.append(pair)

            # ---- x + weights on the SP/HWDGE ring ----
            # ring order: x half0, W1, w2, b1, b2, x half1 (xpool bufs=1)
            x_half = [None, None]
            x_half[0] = xpool.tile([P, TCH // NH, H], f32, tag="x4", name="x4a")
            nc.sync.dma_start(
                out=x_half[0][:],
                in_=x[0:HAL, :].rearrange("(a p) h -> p a h", p=P))
            w1_sb = singles.tile([P, KT, H], f32)  # w1_sb[p,k,m] = W1[k*128+p, m]
            nc.sync.dma_start(out=w1_sb[:], in_=w1.rearrange("(k p) m -> p k m", p=P))
            w2_sb = singles.tile([P, KT, E], f32)  # w2_sb[p,k,e] = W2[k*128+p, e]
            nc.sync.dma_start(out=w2_sb[:], in_=w2.rearrange("(k p) e -> p k e", p=P))
            b1_sb = singles.tile([P, KT], f32)  # b1_sb[p,m] = b1[m*128+p]
            nc.sync.dma_start(out=b1_sb[:], in_=b1.rearrange("(m p) -> p m", p=P))
            b2_sb = singles.tile([E, 1], f32)  # b2 per partition (expert) for stage3
            nc.sync.dma_start(out=b2_sb[:], in_=b2[:, None])
            x_half[1] = xpool.tile([P, TCH // NH, H], f32, tag="x4", name="x4b")
            nc.sync.dma_start(
                out=x_half[1][:],
                in_=x[HAL:T, :].rearrange("(a p) h -> p a h", p=P))
            ident = singles.tile([P, P], f32)
            make_identity(nc, ident[:])

            xT = singles.tile([P, KT, T], f32)  # xT[p,k,t] = x[t, k*128+p]
            hT = singles.tile([P, KT, T], f32)  # hT[p,m,t] = relu(x@W1+b1)[t, m*128+p]
            gates = singles.tile([P, TCH, E], f32)

            # ---- transposes (PE) ----
            for tch in range(TCH):
                x4 = x_half[tch // (TCH // NH)]
                a = tch % (TCH // NH)
                for k in range(KT):
                    pt = psumT.tile([P, P], f32)
                    nc.tensor.transpose(pt[:], x4[:, a, k * P:(k + 1) * P], ident[:])
                    nc.scalar.copy(out=xT[:, k, tch * P:(tch + 1) * P], in_=pt[:])

            for n in range(NH):
                sl = slice(n * HAL, (n + 1) * HAL)
                # ---- stage 2: hT[:, :, half] = relu(W1.T @ xT + b1) ----
                for m in range(KT):
                    ps = psum2.tile([P, HAL], f32)
                    for k in range(KT):
                        nc.tensor.matmul(
                            ps[:],
                            lhsT=w1_sb[:, k, m * P:(m + 1) * P],
                            rhs=xT[:, k, sl],
                            start=(k == 0), stop=(k == KT - 1),
                        )
                    nc.scalar.activation(
                        out=hT[:, m, sl], in_=ps[:],
                        func=mybir.ActivationFunctionType.Relu,
                        bias=b1_sb[:, m:m + 1], scale=1.0,
                    )

                # ---- stage 3: logitsT[e, half] = W2.T @ hT (+ b2) ----
                ps3 = psum3.tile([E, HAL], f32)
                for k in range(KT):
                    nc.tensor.matmul(
                        ps3[:],
                        lhsT=w2_sb[:, k, :],
                        rhs=hT[:, k, sl],
                        start=(k == 0), stop=(k == KT - 1),
                    )
                lT = ltpool.tile([E, HAL], f32, tag="lT")
                nc.scalar.activation(out=lT[:], in_=ps3[:],
                                     func=mybir.ActivationFunctionType.Identity,
                                     bias=b2_sb[:, 0:1], scale=1.0)

                # ---- stage 4 per 128-token chunk: softmax + top-2 gates ----
                for a in range(TCH // NH):
                    tch = n * (TCH // NH) + a
                    pl = psumL.tile([P, E], f32)
                    nc.tensor.transpose(pl[:], lT[:, a * P:(a + 1) * P],
                                        ident[:E, :E])
                    negmax = smalls.tile([P, 1], f32, tag="negmax")
                    nc.vector.reduce_max(negmax[:], pl[:],
                                         axis=mybir.AxisListType.X, negate=True)
                    exps = smalls.tile([P, E], f32, tag="exps")
                    nc.scalar.activation(exps[:], pl[:],
                                         func=mybir.ActivationFunctionType.Exp,
                                         bias=negmax[:], scale=1.0)
                    ssum = smalls.tile([P, 1], f32, tag="ssum")
                    nc.vector.reduce_sum(ssum[:], exps[:],
                                         axis=mybir.AxisListType.X)
                    rs = smalls.tile([P, 1], f32, tag="rs")
                    nc.vector.reciprocal(rs[:], ssum[:])
                    probs = smalls.tile([P, E], f32, tag="probs")
                    nc.vector.tensor_scalar_mul(probs[:], exps[:], rs[:])
                    # top-2: keep the 2 largest probs, zero the rest
                    mx8 = smalls.tile([P, 8], f32, tag="mx8")
                    nc.vector.max(mx8[:], probs[:])
                    nc.vector.memset(mx8[:, 2:], 0.0)
                    zap = smalls.tile([P, E], f32, tag="zap")
                    nc.vector.match_replace(out=zap[:], in_to_replace=mx8[:],
                                            in_values=probs[:], imm_value=0.0)
                    nc.vector.tensor_sub(gates[:, tch, :], probs[:], zap[:])

                # ---- stage 5: weighted combine for this half's chunks ----
                for a in range(TCH // NH):
                    tch = n * (TCH // NH) + a
                    acc = accpool.tile([P, H], f32, tag="acc")
                    first = True
                    for g in range(E // EH):
                        eo_t = eo_tiles[tch][g]
                        for ee in range(EH):
                            e = g * EH + ee
                            if first:
                                nc.vector.tensor_scalar_mul(
                                    acc[:], eo_t[:, ee, :], gates[:, tch, e:e + 1])
                                first = False
                            else:
                                nc.vector.scalar_tensor_tensor(
                                    out=acc[:], in0=eo_t[:, ee, :],
                                    scalar=gates[:, tch, e:e + 1], in1=acc[:],
                                    op0=mybir.AluOpType.mult,
                                    op1=mybir.AluOpType.add,
                                )
                    nc.sync.dma_start(out=out[tch * P:(tch + 1) * P, :], in_=acc[:])

    nc.compile()
    return nc


def _get_nc():
    global _compiled_nc
    if _compiled_nc is None:
        _compiled_nc = _build()
    return _compiled_nc


def kernel(hidden_states, expert_outputs, W1, b1, W2, b2, k=2):
    from concourse.bass_utils import run_bass_kernel_spmd

    hs = np.ascontiguousarray(np.asarray(hidden_states, dtype=np.float32)).reshape(B * S, H)
    eo = np.ascontiguousarray(np.asarray(expert_outputs, dtype=np.float32)).reshape(E, B * S, H)
    w1 = np.ascontiguousarray(np.asarray(W1, dtype=np.float32))
    b1v = np.ascontiguousarray(np.asarray(b1, dtype=np.float32))
    w2 = np.ascontiguousarray(np.asarray(W2, dtype=np.float32))
    b2v = np.ascontiguousarray(np.asarray(b2, dtype=np.float32))

    in_maps = []
    for c in range(N_CORES):
        sl = slice(c * T, (c + 1) * T)
        in_maps.append({
            "x": np.ascontiguousarray(hs[sl]),
            "eo": np.ascontiguousarray(eo[:, sl, :]),
            "w1": w1, "b1": b1v, "w2": w2, "b2": b2v,
        })

    nc = _get_nc()
    res = run_bass_kernel_spmd(nc, in_maps, core_ids=list(range(N_CORES)))
    full = np.concatenate([res.results[c]["out"] for c in range(N_CORES)], axis=0)
    return full.reshape(B, S, H)


# revision 6
# speedup vs baseline: 1.6776x; 1.2009x over previous
"""MoE expert-gating kernel for 8 Trainium2 NeuronCores.

Problem (nn_ExpertGating): router MLP (H->H relu, H->E) + softmax + top-2
gating + weighted combine of per-expert outputs.

Sharding: data-parallel over the B*S=8192 tokens -> 1024 tokens per core.
Each core runs the full router for its tokens and combines its slice of all
8 experts' outputs.  No collectives needed; host concatenates the slices.

Per-core pipeline (T=1024 tokens, H=1024, E=8), fp32 throughout (top-2
selection needs fp32 logits: min top2/3 margin on this data is ~5e-6):
  1. transpose x via PE: xT[h, t]
  2. hT = relu(W1.T @ xT + b1)          (PE fp32 2-pass + ACT)
  3. logitsT[e, t] = W2.T @ hT + b2     (PE, W2 stationary -> tiny LDW;
     b2 folded into the PSUM->SBUF copy where experts sit on partitions)
  4. transpose logits chunks back to [t, E], softmax over E, top-2 via
     max8 + match_replace -> dense gates [t, E]
  5. out[t] = sum_e gates[t, e] * eo[e, t]   (DVE scalar_tensor_tensor)

Stage 2-4 run per 512-token half so the first half's combines overlap the
second half's router matmuls.  eo loads stream on the gpsimd (SWDGE) ring
from t=0; x/W/out live on the SP (HWDGE) ring.
"""

import numpy as np

B, S, H, E = 4, 2048, 1024, 8
N_CORES = 8
T = (B * S) // N_CORES  # tokens per core
P = 128  # partitions
TCH = T // P  # token chunks per core (8)
KT = H // P  # contraction tiles (8)
NH = 2  # halves of the token range for stage 2/3
HAL = T // NH  # 512
EH = E // 2  # experts per eo tile (SBUF budget)

_compiled_nc = None


def _build():
    import concourse.bacc as bacc
    import concourse.bass as bass
    import concourse.tile as tile
    from concourse import mybir
    from concourse.masks import make_identity

    f32 = mybir.dt.float32
    nc = bacc.Bacc("TRN2", target_bir_lowering=False, debug=False,
                   num_devices=N_CORES)

    x = nc.dram_tensor("x", [T, H], f32, kind="ExternalInput").ap()
    eo = nc.dram_tensor("eo", [E * T, H], f32, kind="ExternalInput").ap()
    w1 = nc.dram_tensor("w1", [H, H], f32, kind="ExternalInput").ap()
    b1 = nc.dram_tensor("b1", [H], f32, kind="ExternalInput").ap()
    w2 = nc.dram_tensor("w2", [H, E], f32, kind="ExternalInput").ap()
    b2 = nc.dram_tensor("b2", [E], f32, kind="ExternalInput").ap()
    out = nc.dram_tensor("out", [T, H], f32, kind="ExternalOutput").ap()

    with tile.TileContext(nc) as tc:
        with (
            tc.tile_pool(name="singles", bufs=1) as singles,
            tc.tile_pool(name="xpool", bufs=1) as xpool,
            tc.tile_pool(name="eopool", bufs=4) as eopool,
            tc.tile_pool(name="accpool", bufs=3) as accpool,
            tc.tile_pool(name="smalls", bufs=8) as smalls,
            tc.tile_pool(name="ltpool", bufs=2) as ltpool,
            tc.tile_pool(name="psumT", bufs=2, space="PSUM") as psumT,
            tc.tile_pool(name="psum2", bufs=2, space="PSUM") as psum2,
            tc.tile_pool(name="psum3", bufs=2, space="PSUM") as psum3,
            tc.tile_pool(name="psumL", bufs=2, space="PSUM") as psumL,
        ):
            # ---- x + weights on the SP/HWDGE ring ----
            # ring order: x half0, W1, w2, b1, b2, x half1 (xpool bufs=1)
            x_half = [None, None]
            x_half[0] = xpool.tile([P, TCH // NH, H], f32, tag="x4", name="x4a")
            nc.sync.dma_start(
                out=x_half[0][:],
                in_=x[0:HAL, :].rearrange("(a p) h -> p a h", p=P))
            w1_sb = singles.tile([P, KT, H], f32)  # w1_sb[p,k,m] = W1[k*128+p, m]
            nc.sync.dma_start(out=w1_sb[:], in_=w1.rearrange("(k p) m -> p k m", p=P))
            w2_sb = singles.tile([P, KT, E], f32)  # w2_sb[p,k,e] = W2[k*128+p, e]
            nc.sync.dma_start(out=w2_sb[:], in_=w2.rearrange("(k p) e -> p k e", p=P))
            b1_sb = singles.tile([P, KT], f32)  # b1_sb[p,m] = b1[m*128+p]
            nc.sync.dma_start(out=b1_sb[:], in_=b1.rearrange("(m p) -> p m", p=P))
            b2_sb = singles.tile([E, 1], f32)  # b2 per partition (expert) for stage3
            nc.sync.dma_start(out=b2_sb[:], in_=b2[:, None])
            x_half[1] = xpool.tile([P, TCH // NH, H], f32, tag="x4", name="x4b")
            nc.sync.dma_start(
                out=x_half[1][:],
                in_=x[HAL:T, :].rearrange("(a p) h -> p a h", p=P))
            ident = singles.tile([P, P], f32)
            make_identity(nc, ident[:])
            iota_u = singles.tile([P, 1], mybir.dt.uint32)
            nc.gpsimd.iota(iota_u[:], pattern=[[0, 1]], base=0,
                           channel_multiplier=1)

            xT = singles.tile([P, KT, T], f32)  # xT[p,k,t] = x[t, k*128+p]
            hT = singles.tile([P, KT, T], f32)  # hT[p,m,t] = relu(x@W1+b1)[t, m*128+p]

            # ---- transposes (PE) ----
            for tch in range(TCH):
                x4 = x_half[tch // (TCH // NH)]
                a = tch % (TCH // NH)
                for k in range(KT):
                    pt = psumT.tile([P, P], f32)
                    nc.tensor.transpose(pt[:], x4[:, a, k * P:(k + 1) * P], ident[:])
                    nc.scalar.copy(out=xT[:, k, tch * P:(tch + 1) * P], in_=pt[:])

            for n in range(NH):
                sl = slice(n * HAL, (n + 1) * HAL)
                # ---- stage 2: hT[:, :, half] = relu(W1.T @ xT + b1) ----
                for m in range(KT):
                    ps = psum2.tile([P, HAL], f32)
                    for k in range(KT):
                        nc.tensor.matmul(
                            ps[:],
                            lhsT=w1_sb[:, k, m * P:(m + 1) * P],
                            rhs=xT[:, k, sl],
                            start=(k == 0), stop=(k == KT - 1),
                        )
                    nc.scalar.activation(
                        out=hT[:, m, sl], in_=ps[:],
                        func=mybir.ActivationFunctionType.Relu,
                        bias=b1_sb[:, m:m + 1], scale=1.0,
                    )

                # ---- stage 3: logitsT[e, half] = W2.T @ hT (+ b2) ----
                ps3 = psum3.tile([E, HAL], f32)
                for k in range(KT):
                    nc.tensor.matmul(
                        ps3[:],
                        lhsT=w2_sb[:, k, :],
                        rhs=hT[:, k, sl],
                        start=(k == 0), stop=(k == KT - 1),
                    )
                lT = ltpool.tile([E, HAL], f32, tag="lT")
                nc.scalar.activation(out=lT[:], in_=ps3[:],
                                     func=mybir.ActivationFunctionType.Identity,
                                     bias=b2_sb[:, 0:1], scale=1.0)

                # ---- stage 4+5 per 128-token chunk: softmax, top-2,
                # indirect gather of the 2 selected expert rows, combine ----
                for a in range(TCH // NH):
                    tch = n * (TCH // NH) + a
                    pl = psumL.tile([P, E], f32)
                    nc.tensor.transpose(pl[:], lT[:, a * P:(a + 1) * P],
                                        ident[:E, :E])
                    negmax = smalls.tile([P, 1], f32, tag="negmax")
                    nc.vector.reduce_max(negmax[:], pl[:],
                                         axis=mybir.AxisListType.X, negate=True)
                    exps = smalls.tile([P, E], f32, tag="exps")
                    nc.scalar.activation(exps[:], pl[:],
                                         func=mybir.ActivationFunctionType.Exp,
                                         bias=negmax[:], scale=1.0)
                    ssum = smalls.tile([P, 1], f32, tag="ssum")
                    nc.vector.reduce_sum(ssum[:], exps[:],
                                         axis=mybir.AxisListType.X)
                    rs = smalls.tile([P, 1], f32, tag="rs")
                    nc.vector.reciprocal(rs[:], ssum[:])
                    # top-2 of exps == top-2 of probs (positive scale);
                    # gate value = exp * (1/sum)
                    mx8 = smalls.tile([P, 8], f32, tag="mx8")
                    nc.vector.max(mx8[:], exps[:])
                    idx8 = smalls.tile([P, 8], mybir.dt.uint32, tag="idx8")
                    nc.vector.max_index(idx8[:], mx8[:], exps[:])
                    # flat eo row = expert*T + (tch*128 + partition)
                    base = smalls.tile([P, 1], mybir.dt.uint32, tag="base")
                    nc.vector.tensor_scalar_add(base[:], iota_u[:], tch * P)
                    rows = smalls.tile([P, 2], mybir.dt.uint32, tag="rows")
                    for s in range(2):
                        nc.vector.tensor_scalar(
                            rows[:, s:s + 1], idx8[:, s:s + 1],
                            scalar1=T, scalar2=None,
                            op0=mybir.AluOpType.mult)
                        nc.vector.tensor_tensor(
                            out=rows[:, s:s + 1], in0=rows[:, s:s + 1],
                            in1=base[:], op=mybir.AluOpType.add)
                    eo_g0 = eopool.tile([P, H], f32, tag="eog0")
                    nc.gpsimd.indirect_dma_start(
                        out=eo_g0[:], out_offset=None, in_=eo,
                        in_offset=bass.IndirectOffsetOnAxis(
                            ap=rows[:, 0:1], axis=0))
                    eo_g1 = eopool.tile([P, H], f32, tag="eog1")
                    nc.gpsimd.indirect_dma_start(
                        out=eo_g1[:], out_offset=None, in_=eo,
                        in_offset=bass.IndirectOffsetOnAxis(
                            ap=rows[:, 1:2], axis=0))
                    g0 = smalls.tile([P, 1], f32, tag="g0")
                    nc.vector.tensor_mul(g0[:], mx8[:, 0:1], rs[:])
                    g1 = smalls.tile([P, 1], f32, tag="g1")
                    nc.vector.tensor_mul(g1[:], mx8[:, 1:2], rs[:])
                    acc = accpool.tile([P, H], f32, tag="acc")
                    nc.vector.tensor_scalar_mul(acc[:], eo_g0[:], g0[:])
                    nc.vector.scalar_tensor_tensor(
                        out=acc[:], in0=eo_g1[:], scalar=g1[:], in1=acc[:],
                        op0=mybir.AluOpType.mult, op1=mybir.AluOpType.add)
                    nc.sync.dma_start(out=out[tch * P:(tch + 1) * P, :],
                                      in_=acc[:])

    nc.compile()
    return nc


def _get_nc():
    global _compiled_nc
    if _compiled_nc is None:
        _compiled_nc = _build()
    return _compiled_nc


def kernel(hidden_states, expert_outputs, W1, b1, W2, b2, k=2):
    from concourse.bass_utils import run_bass_kernel_spmd

    hs = np.ascontiguousarray(np.asarray(hidden_states, dtype=np.float32)).reshape(B * S, H)
    eo = np.ascontiguousarray(np.asarray(expert_outputs, dtype=np.float32)).reshape(E, B * S, H)
    w1 = np.ascontiguousarray(np.asarray(W1, dtype=np.float32))
    b1v = np.ascontiguousarray(np.asarray(b1, dtype=np.float32))
    w2 = np.ascontiguousarray(np.asarray(W2, dtype=np.float32))
    b2v = np.ascontiguousarray(np.asarray(b2, dtype=np.float32))

    in_maps = []
    for c in range(N_CORES):
        sl = slice(c * T, (c + 1) * T)
        in_maps.append({
            "x": np.ascontiguousarray(hs[sl]),
            "eo": np.ascontiguousarray(eo[:, sl, :]).reshape(E * T, H),
            "w1": w1, "b1": b1v, "w2": w2, "b2": b2v,
        })

    nc = _get_nc()
    res = run_bass_kernel_spmd(nc, in_maps, core_ids=list(range(N_CORES)))
    full = np.concatenate([res.results[c]["out"] for c in range(N_CORES)], axis=0)
    return full.reshape(B, S, H)


# revision 8
# speedup vs baseline: 1.8699x; 1.1147x over previous
"""MoE expert-gating kernel for 8 Trainium2 NeuronCores.

Problem (nn_ExpertGating): router MLP (H->H relu, H->E) + softmax + top-2
gating + weighted combine of per-expert outputs.

Sharding: data-parallel over the B*S=8192 tokens -> 1024 tokens per core.
Each core runs the full router for its tokens and combines its slice of all
8 experts' outputs.  No collectives needed; host concatenates the slices.

Per-core pipeline (T=1024 tokens, H=1024, E=8), fp32 throughout (top-2
selection needs fp32 logits: min top2/3 margin on this data is ~5e-6):
  1. transpose x via PE: xT[h, t]
  2. hT = relu(W1.T @ xT + b1)          (PE fp32 2-pass + ACT)
  3. logitsT[e, t] = W2.T @ hT + b2     (PE, W2 stationary -> tiny LDW;
     b2 folded into the PSUM->SBUF copy where experts sit on partitions)
  4. transpose logits chunks back to [t, E], softmax over E, top-2 via
     max8 + match_replace -> dense gates [t, E]
  5. out[t] = sum_e gates[t, e] * eo[e, t]   (DVE scalar_tensor_tensor)

Stage 2-4 run per 512-token half so the first half's combines overlap the
second half's router matmuls.  eo loads stream on the gpsimd (SWDGE) ring
from t=0; x/W/out live on the SP (HWDGE) ring.
"""

import numpy as np

B, S, H, E = 4, 2048, 1024, 8
N_CORES = 8
T = (B * S) // N_CORES  # tokens per core
P = 128  # partitions
TCH = T // P  # token chunks per core (8)
KT = H // P  # contraction tiles (8)
NH = 2  # halves of the token range for stage 2/3
HAL = T // NH  # 512
EH = E // 2  # experts per eo tile (SBUF budget)

_compiled_nc = None


def _build():
    import concourse.bacc as bacc
    import concourse.bass as bass
    import concourse.tile as tile
    from concourse import mybir
    from concourse.masks import make_identity

    f32 = mybir.dt.float32
    nc = bacc.Bacc("TRN2", target_bir_lowering=False, debug=False,
                   num_devices=N_CORES)

    x = nc.dram_tensor("x", [T, H], f32, kind="ExternalInput").ap()
    eo = nc.dram_tensor("eo", [E * T, H], f32, kind="ExternalInput").ap()
    f16 = mybir.dt.float16
    w1h = nc.dram_tensor("w1h", [H, H], f16, kind="ExternalInput").ap()
    w1l = nc.dram_tensor("w1l", [H, H], f16, kind="ExternalInput").ap()
    b1 = nc.dram_tensor("b1", [H], f32, kind="ExternalInput").ap()
    w2 = nc.dram_tensor("w2", [H, E], f32, kind="ExternalInput").ap()
    b2 = nc.dram_tensor("b2", [E], f32, kind="ExternalInput").ap()
    out = nc.dram_tensor("out", [T, H], f32, kind="ExternalOutput").ap()

    with tile.TileContext(nc) as tc:
        with (
            tc.tile_pool(name="singles", bufs=1) as singles,
            tc.tile_pool(name="xpool", bufs=1) as xpool,
            tc.tile_pool(name="eopool", bufs=4) as eopool,
            tc.tile_pool(name="accpool", bufs=3) as accpool,
            tc.tile_pool(name="smalls", bufs=8) as smalls,
            tc.tile_pool(name="ltpool", bufs=2) as ltpool,
            tc.tile_pool(name="tmppool", bufs=3) as tmppool,
            tc.tile_pool(name="hprepool", bufs=2) as hprepool,
            tc.tile_pool(name="psumT", bufs=2, space="PSUM") as psumT,
            tc.tile_pool(name="psum2", bufs=2, space="PSUM") as psum2,
            tc.tile_pool(name="psum2c", bufs=2, space="PSUM") as psum2c,
            tc.tile_pool(name="psum3", bufs=1, space="PSUM") as psum3,
        ):
            # ---- x + weights on the SP/HWDGE ring ----
            # ring order: x half0, W1, w2, b1, b2, x half1 (xpool bufs=1)
            x_half = [None, None]
            x_half[0] = xpool.tile([P, TCH // NH, H], f32, tag="x4", name="x4a")
            nc.sync.dma_start(
                out=x_half[0][:],
                in_=x[0:HAL, :].rearrange("(a p) h -> p a h", p=P))
            w1h_sb = singles.tile([P, KT, H], f16)  # fp16 high half of W1
            nc.sync.dma_start(out=w1h_sb[:], in_=w1h.rearrange("(k p) m -> p k m", p=P))
            w1l_sb = singles.tile([P, KT, H], f16)  # fp16 low half of W1, x8192
            nc.sync.dma_start(out=w1l_sb[:], in_=w1l.rearrange("(k p) m -> p k m", p=P))
            w2_sb = singles.tile([P, KT, E], f32)  # w2_sb[p,k,e] = W2[k*128+p, e]
            nc.sync.dma_start(out=w2_sb[:], in_=w2.rearrange("(k p) e -> p k e", p=P))
            b1_sb = singles.tile([P, KT], f32)  # b1_sb[p,m] = b1[m*128+p]
            nc.sync.dma_start(out=b1_sb[:], in_=b1.rearrange("(m p) -> p m", p=P))
            b2_sb = singles.tile([E, 1], f32)  # b2 per partition (expert) for stage3
            nc.sync.dma_start(out=b2_sb[:], in_=b2[:, None])
            x_half[1] = xpool.tile([P, TCH // NH, H], f32, tag="x4", name="x4b")
            nc.sync.dma_start(
                out=x_half[1][:],
                in_=x[HAL:T, :].rearrange("(a p) h -> p a h", p=P))
            ident = singles.tile([P, P], f32)
            make_identity(nc, ident[:])
            iota_u = singles.tile([P, 1], mybir.dt.uint32)
            nc.gpsimd.iota(iota_u[:], pattern=[[0, 1]], base=0,
                           channel_multiplier=1)

            xThi = singles.tile([P, KT, T], f16)  # fp16 high half of x^T
            xTlo = singles.tile([P, KT, T], f16)  # fp16 low half of x^T, x8192
            hT = singles.tile([P, KT, T], f32)  # hT[p,m,t] = relu(x@W1+b1)[t, m*128+p]

            # ---- transposes (PE) + fp16 hi/lo split of x^T ----
            for tch in range(TCH):
                x4 = x_half[tch // (TCH // NH)]
                a = tch % (TCH // NH)
                csl = slice(tch * P, (tch + 1) * P)
                for k in range(KT):
                    pt = psumT.tile([P, P], f32, tag="pt")
                    nc.tensor.transpose(pt[:], x4[:, a, k * P:(k + 1) * P], ident[:])
                    nc.scalar.copy(out=xThi[:, k, csl], in_=pt[:])
                    tmp = tmppool.tile([P, P], f32, tag="tmp")
                    nc.vector.tensor_tensor(out=tmp[:], in0=pt[:],
                                            in1=xThi[:, k, csl],
                                            op=mybir.AluOpType.subtract)
                    nc.vector.tensor_scalar_mul(xTlo[:, k, csl], tmp[:], 8192.0)

            for n in range(NH):
                sl = slice(n * HAL, (n + 1) * HAL)
                # ---- stage 2: hT = relu(W1.T @ xT + b1), fp16 x3 passes:
                # main = xhi*whi ; corr = (xhi*wlo' + xlo'*whi) with both lo
                # terms pre-scaled by 2^13 -> hT = relu(main + corr/2^13 + b1)
                for m in range(KT):
                    msl = slice(m * P, (m + 1) * P)
                    ps = psum2.tile([P, HAL], f32)
                    for k in range(KT):
                        nc.tensor.matmul(
                            ps[:], lhsT=w1h_sb[:, k, msl], rhs=xThi[:, k, sl],
                            start=(k == 0), stop=(k == KT - 1),
                        )
                    psc = psum2c.tile([P, HAL], f32)
                    for k in range(KT):
                        nc.tensor.matmul(
                            psc[:], lhsT=w1l_sb[:, k, msl], rhs=xThi[:, k, sl],
                            start=(k == 0), stop=False,
                        )
                        nc.tensor.matmul(
                            psc[:], lhsT=w1h_sb[:, k, msl], rhs=xTlo[:, k, sl],
                            start=False, stop=(k == KT - 1),
                        )
                    hcorr = hprepool.tile([P, HAL], f32, tag="hcorr")
                    nc.scalar.mul(out=hcorr[:], in_=psc[:], mul=1.0 / 8192.0)
                    hpre = hprepool.tile([P, HAL], f32, tag="hpre")
                    nc.vector.tensor_tensor(out=hpre[:], in0=ps[:], in1=hcorr[:],
                                            op=mybir.AluOpType.add)
                    nc.scalar.activation(
                        out=hT[:, m, sl], in_=hpre[:],
                        func=mybir.ActivationFunctionType.Relu,
                        bias=b1_sb[:, m:m + 1], scale=1.0,
                    )

                # ---- stage 3: logitsT[e, half] = W2.T @ hT (+ b2) ----
                ps3 = psum3.tile([E, HAL], f32)
                for k in range(KT):
                    nc.tensor.matmul(
                        ps3[:],
                        lhsT=w2_sb[:, k, :],
                        rhs=hT[:, k, sl],
                        start=(k == 0), stop=(k == KT - 1),
                    )
                lT = ltpool.tile([E, HAL], f32, tag="lT")
                nc.scalar.activation(out=lT[:], in_=ps3[:],
                                     func=mybir.ActivationFunctionType.Identity,
                                     bias=b2_sb[:, 0:1], scale=1.0)

                # ---- stage 4+5 per 128-token chunk: softmax, top-2,
                # indirect gather of the 2 selected expert rows, combine ----
                for a in range(TCH // NH):
                    tch = n * (TCH // NH) + a
                    pl = psumT.tile([P, P], f32, tag="pt", name="pl")
                    nc.tensor.transpose(pl[:, :E], lT[:, a * P:(a + 1) * P],
                                        ident[:E, :E])
                    negmax = smalls.tile([P, 1], f32, tag="negmax")
                    nc.vector.reduce_max(negmax[:], pl[:, :E],
                                         axis=mybir.AxisListType.X, negate=True)
                    exps = smalls.tile([P, E], f32, tag="exps")
                    nc.scalar.activation(exps[:], pl[:, :E],
                                         func=mybir.ActivationFunctionType.Exp,
                                         bias=negmax[:], scale=1.0)
                    ssum = smalls.tile([P, 1], f32, tag="ssum")
                    nc.vector.reduce_sum(ssum[:], exps[:],
                                         axis=mybir.AxisListType.X)
                    rs = smalls.tile([P, 1], f32, tag="rs")
                    nc.vector.reciprocal(rs[:], ssum[:])
                    # top-2 of exps == top-2 of probs (positive scale);
                    # gate value = exp * (1/sum)
                    mx8 = smalls.tile([P, 8], f32, tag="mx8")
                    nc.vector.max(mx8[:], exps[:])
                    idx8 = smalls.tile([P, 8], mybir.dt.uint32, tag="idx8")
                    nc.vector.max_index(idx8[:], mx8[:], exps[:])
                    # flat eo row = expert*T + (tch*128 + partition)
                    base = smalls.tile([P, 1], mybir.dt.uint32, tag="base")
                    nc.vector.tensor_scalar_add(base[:], iota_u[:], tch * P)
                    rows = smalls.tile([P, 2], mybir.dt.uint32, tag="rows")
                    for s in range(2):
                        nc.vector.tensor_scalar(
                            rows[:, s:s + 1], idx8[:, s:s + 1],
                            scalar1=T, scalar2=None,
                            op0=mybir.AluOpType.mult)
                        nc.vector.tensor_tensor(
                            out=rows[:, s:s + 1], in0=rows[:, s:s + 1],
                            in1=base[:], op=mybir.AluOpType.add)
                    eo_g0 = eopool.tile([P, H], f32, tag="eog0")
                    nc.gpsimd.indirect_dma_start(
                        out=eo_g0[:], out_offset=None, in_=eo,
                        in_offset=bass.IndirectOffsetOnAxis(
                            ap=rows[:, 0:1], axis=0))
                    eo_g1 = eopool.tile([P, H], f32, tag="eog1")
                    nc.gpsimd.indirect_dma_start(
                        out=eo_g1[:], out_offset=None, in_=eo,
                        in_offset=bass.IndirectOffsetOnAxis(
                            ap=rows[:, 1:2], axis=0))
                    g0 = smalls.tile([P, 1], f32, tag="g0")
                    nc.vector.tensor_mul(g0[:], mx8[:, 0:1], rs[:])
                    g1 = smalls.tile([P, 1], f32, tag="g1")
                    nc.vector.tensor_mul(g1[:], mx8[:, 1:2], rs[:])
                    acc = accpool.tile([P, H], f32, tag="acc")
                    nc.vector.tensor_scalar_mul(acc[:], eo_g0[:], g0[:])
                    nc.vector.scalar_tensor_tensor(
                        out=acc[:], in0=eo_g1[:], scalar=g1[:], in1=acc[:],
                        op0=mybir.AluOpType.mult, op1=mybir.AluOpType.add)
                    nc.sync.dma_start(out=out[tch * P:(tch + 1) * P, :],
                                      in_=acc[:])

    nc.compile()
    return nc


def _get_nc():
    global _compiled_nc
    if _compiled_nc is None:
        _compiled_nc = _build()
    return _compiled_nc


def kernel(hidden_states, expert_outputs, W1, b1, W2, b2, k=2):
    from concourse.bass_utils import run_bass_kernel_spmd

    hs = np.ascontiguousarray(np.asarray(hidden_states, dtype=np.float32)).reshape(B * S, H)
    eo = np.ascontiguousarray(np.asarray(expert_outputs, dtype=np.float32)).reshape(E, B * S, H)
    w1f = np.asarray(W1, dtype=np.float32)
    w1h = np.ascontiguousarray(w1f.astype(np.float16))
    w1l = np.ascontiguousarray(
        ((w1f.astype(np.float64) - w1h.astype(np.float64)) * 8192.0)
        .astype(np.float16))
    b1v = np.ascontiguousarray(np.asarray(b1, dtype=np.float32))
    w2 = np.ascontiguousarray(np.asarray(W2, dtype=np.float32))
    b2v = np.ascontiguousarray(np.asarray(b2, dtype=np.float32))

    in_maps = []
    for c in range(N_CORES):
        sl = slice(c * T, (c + 1) * T)
        in_maps.append({
            "x": np.ascontiguousarray(hs[sl]),
            "eo": np.ascontiguousarray(eo[:, sl, :]).reshape(E * T, H),
            "w1h": w1h, "w1l": w1l, "b1": b1v, "w2": w2, "b2": b2v,
        })

    nc = _get_nc()
    res = run_bass_kernel_spmd(nc, in_maps, core_ids=list(range(N_CORES)))
    full = np.concatenate([res.results[c]["out"] for c in range(N_CORES)], axis=0)
    return full.reshape(B, S, H)


# revision 10
# speedup vs baseline: 1.9791x; 1.0584x over previous
"""MoE expert-gating kernel for 8 Trainium2 NeuronCores.

Problem (nn_ExpertGating): router MLP (H->H relu, H->E) + softmax + top-2
gating + weighted combine of per-expert outputs.

Sharding: data-parallel over the B*S=8192 tokens -> 1024 tokens per core.
Each core runs the full router for its tokens and combines its slice of all
8 experts' outputs.  No collectives needed; host concatenates the slices.

Per-core pipeline (T=1024 tokens, H=1024, E=8), fp32 throughout (top-2
selection needs fp32 logits: min top2/3 margin on this data is ~5e-6):
  1. transpose x via PE: xT[h, t]
  2. hT = relu(W1.T @ xT + b1)          (PE fp32 2-pass + ACT)
  3. logitsT[e, t] = W2.T @ hT + b2     (PE, W2 stationary -> tiny LDW;
     b2 folded into the PSUM->SBUF copy where experts sit on partitions)
  4. transpose logits chunks back to [t, E], softmax over E, top-2 via
     max8 + match_replace -> dense gates [t, E]
  5. out[t] = sum_e gates[t, e] * eo[e, t]   (DVE scalar_tensor_tensor)

Stage 2-4 run per 512-token half so the first half's combines overlap the
second half's router matmuls.  eo loads stream on the gpsimd (SWDGE) ring
from t=0; x/W/out live on the SP (HWDGE) ring.
"""

import numpy as np

B, S, H, E = 4, 2048, 1024, 8
N_CORES = 8
T = (B * S) // N_CORES  # tokens per core
P = 128  # partitions
TCH = T // P  # token chunks per core (8)
KT = H // P  # contraction tiles (8)
NH = 2  # halves of the token range for stage 2/3
HAL = T // NH  # 512
EH = E // 2  # experts per eo tile (SBUF budget)

_compiled_nc = None


def _build():
    import concourse.bacc as bacc
    import concourse.bass as bass
    import concourse.tile as tile
    from concourse import mybir
    from concourse.masks import make_identity

    f32 = mybir.dt.float32
    nc = bacc.Bacc("TRN2", target_bir_lowering=False, debug=False,
                   num_devices=N_CORES)

    x = nc.dram_tensor("x", [T, H], f32, kind="ExternalInput").ap()
    eo = nc.dram_tensor("eo", [E * T, H], f32, kind="ExternalInput").ap()
    f16 = mybir.dt.float16
    w1h = nc.dram_tensor("w1h", [H, H], f16, kind="ExternalInput").ap()
    w1l = nc.dram_tensor("w1l", [H, H], f16, kind="ExternalInput").ap()
    b1 = nc.dram_tensor("b1", [H], f32, kind="ExternalInput").ap()
    w2 = nc.dram_tensor("w2", [H, E], f32, kind="ExternalInput").ap()
    b2 = nc.dram_tensor("b2", [E], f32, kind="ExternalInput").ap()
    out = nc.dram_tensor("out", [T, H], f32, kind="ExternalOutput").ap()

    with tile.TileContext(nc) as tc:
        with (
            tc.tile_pool(name="singles", bufs=1) as singles,
            tc.tile_pool(name="xpool", bufs=1) as xpool,
            tc.tile_pool(name="eopool", bufs=4) as eopool,
            tc.tile_pool(name="accpool", bufs=3) as accpool,
            tc.tile_pool(name="smalls", bufs=8) as smalls,
            tc.tile_pool(name="ltpool", bufs=2) as ltpool,
            tc.tile_pool(name="tmppool", bufs=3) as tmppool,
            tc.tile_pool(name="hprepool", bufs=2) as hprepool,
            tc.tile_pool(name="psumT", bufs=2, space="PSUM") as psumT,
            tc.tile_pool(name="psum2", bufs=2, space="PSUM") as psum2,
            tc.tile_pool(name="psum2c", bufs=2, space="PSUM") as psum2c,
            tc.tile_pool(name="psum3", bufs=1, space="PSUM") as psum3,
            tc.tile_pool(name="psumL", bufs=1, space="PSUM") as psumL,
        ):
            # ---- x + weights on the SP/HWDGE ring ----
            # ring order: x half0, W1, w2, b1, b2, x half1 (xpool bufs=1)
            x_half = [None, None]
            x_half[0] = xpool.tile([P, TCH // NH, H], f32, tag="x4", name="x4a")
            nc.sync.dma_start(
                out=x_half[0][:],
                in_=x[0:HAL, :].rearrange("(a p) h -> p a h", p=P))
            w1h_sb = singles.tile([P, KT, H], f16)  # fp16 high half of W1
            nc.sync.dma_start(out=w1h_sb[:], in_=w1h.rearrange("(k p) m -> p k m", p=P))
            w1l_sb = singles.tile([P, KT, H], f16)  # fp16 low half of W1, x8192
            nc.sync.dma_start(out=w1l_sb[:], in_=w1l.rearrange("(k p) m -> p k m", p=P))
            w2_sb = singles.tile([P, KT, E], f32)  # w2_sb[p,k,e] = W2[k*128+p, e]
            nc.sync.dma_start(out=w2_sb[:], in_=w2.rearrange("(k p) e -> p k e", p=P))
            b1_sb = singles.tile([P, KT], f32)  # b1_sb[p,m] = b1[m*128+p]
            nc.sync.dma_start(out=b1_sb[:], in_=b1.rearrange("(m p) -> p m", p=P))
            b2_sb = singles.tile([E, 1], f32)  # b2 per partition (expert) for stage3
            nc.sync.dma_start(out=b2_sb[:], in_=b2[:, None])
            x_half[1] = xpool.tile([P, TCH // NH, H], f32, tag="x4", name="x4b")
            nc.sync.dma_start(
                out=x_half[1][:],
                in_=x[HAL:T, :].rearrange("(a p) h -> p a h", p=P))
            ident = singles.tile([P, P], f32)
            make_identity(nc, ident[:])
            iota_u = singles.tile([P, 1], mybir.dt.uint32)
            nc.gpsimd.iota(iota_u[:], pattern=[[0, 1]], base=0,
                           channel_multiplier=1)

            xThi = singles.tile([P, KT, T], f16)  # fp16 high half of x^T
            xTlo = singles.tile([P, KT, T], f16)  # fp16 low half of x^T, x8192
            hT = singles.tile([P, KT, T], f32)  # hT[p,m,t] = relu(x@W1+b1)[t, m*128+p]

            # ---- transposes (PE) + fp16 hi/lo split of x^T ----
            for tch in range(TCH):
                x4 = x_half[tch // (TCH // NH)]
                a = tch % (TCH // NH)
                csl = slice(tch * P, (tch + 1) * P)
                for kk in range(KT // 4):
                    pt = psumT.tile([P, 4 * P], f32, tag="pt")
                    for j in range(4):
                        k = kk * 4 + j
                        nc.tensor.transpose(pt[:, j * P:(j + 1) * P],
                                            x4[:, a, k * P:(k + 1) * P],
                                            ident[:])
                    ksl = slice(kk * 4, (kk + 1) * 4)
                    pt3 = pt[:].rearrange("p (j c) -> p j c", j=4)
                    nc.scalar.copy(out=xThi[:, ksl, csl], in_=pt3)
                    tmp = tmppool.tile([P, 4, P], f32, tag="tmp")
                    nc.vector.tensor_tensor(out=tmp[:], in0=pt3,
                                            in1=xThi[:, ksl, csl],
                                            op=mybir.AluOpType.subtract)
                    nc.vector.tensor_scalar_mul(xTlo[:, ksl, csl], tmp[:], 8192.0)

            for n in range(NH):
                sl = slice(n * HAL, (n + 1) * HAL)
                # ---- stage 2: hT = relu(W1.T @ xT + b1), fp16 x3 passes:
                # main = xhi*whi ; corr = (xhi*wlo' + xlo'*whi) with both lo
                # terms pre-scaled by 2^13 -> hT = relu(main + corr/2^13 + b1)
                for m in range(KT):
                    msl = slice(m * P, (m + 1) * P)
                    ps = psum2.tile([P, HAL], f32)
                    for k in range(KT):
                        nc.tensor.matmul(
                            ps[:], lhsT=w1h_sb[:, k, msl], rhs=xThi[:, k, sl],
                            start=(k == 0), stop=(k == KT - 1),
                        )
                    psc = psum2c.tile([P, HAL], f32)
                    for k in range(KT):
                        nc.tensor.matmul(
                            psc[:], lhsT=w1l_sb[:, k, msl], rhs=xThi[:, k, sl],
                            start=(k == 0), stop=False,
                        )
                        nc.tensor.matmul(
                            psc[:], lhsT=w1h_sb[:, k, msl], rhs=xTlo[:, k, sl],
                            start=False, stop=(k == KT - 1),
                        )
                    hcorr = hprepool.tile([P, HAL], f32, tag="hcorr")
                    nc.scalar.mul(out=hcorr[:], in_=psc[:], mul=1.0 / 8192.0)
                    hpre = hprepool.tile([P, HAL], f32, tag="hpre")
                    nc.vector.tensor_tensor(out=hpre[:], in0=ps[:], in1=hcorr[:],
                                            op=mybir.AluOpType.add)
                    nc.scalar.activation(
                        out=hT[:, m, sl], in_=hpre[:],
                        func=mybir.ActivationFunctionType.Relu,
                        bias=b1_sb[:, m:m + 1], scale=1.0,
                    )

                # ---- stage 3: logitsT[e, half] = W2.T @ hT (+ b2) ----
                ps3 = psum3.tile([E, HAL], f32)
                for k in range(KT):
                    nc.tensor.matmul(
                        ps3[:],
                        lhsT=w2_sb[:, k, :],
                        rhs=hT[:, k, sl],
                        start=(k == 0), stop=(k == KT - 1),
                    )
                lT = ltpool.tile([E, HAL], f32, tag="lT")
                nc.scalar.activation(out=lT[:], in_=ps3[:],
                                     func=mybir.ActivationFunctionType.Identity,
                                     bias=b2_sb[:, 0:1], scale=1.0)

                # ---- stage 4+5 per 128-token chunk: softmax, top-2,
                # indirect gather of the 2 selected expert rows, combine ----
                for a in range(TCH // NH):
                    tch = n * (TCH // NH) + a
                    pl = psumL.tile([P, E], f32, tag="pl", name="pl")
                    nc.tensor.transpose(pl[:], lT[:, a * P:(a + 1) * P],
                                        ident[:E, :E])
                    negmax = smalls.tile([P, 1], f32, tag="negmax")
                    nc.vector.reduce_max(negmax[:], pl[:],
                                         axis=mybir.AxisListType.X, negate=True)
                    exps = smalls.tile([P, E], f32, tag="exps")
                    nc.scalar.activation(exps[:], pl[:],
                                         func=mybir.ActivationFunctionType.Exp,
                                         bias=negmax[:], scale=1.0)
                    ssum = smalls.tile([P, 1], f32, tag="ssum")
                    nc.vector.reduce_sum(ssum[:], exps[:],
                                         axis=mybir.AxisListType.X)
                    rs = smalls.tile([P, 1], f32, tag="rs")
                    nc.vector.reciprocal(rs[:], ssum[:])
                    # top-2 of exps == top-2 of probs (positive scale);
                    # gate value = exp * (1/sum)
                    mx8 = smalls.tile([P, 8], f32, tag="mx8")
                    nc.vector.max(mx8[:], exps[:])
                    idx8 = smalls.tile([P, 8], mybir.dt.uint32, tag="idx8")
                    nc.vector.max_index(idx8[:], mx8[:], exps[:])
                    # flat eo row = expert*T + (tch*128 + partition)
                    base = smalls.tile([P, 1], mybir.dt.uint32, tag="base")
                    nc.vector.tensor_scalar_add(base[:], iota_u[:], tch * P)
                    rows = smalls.tile([P, 2], mybir.dt.uint32, tag="rows")
                    for s in range(2):
                        nc.vector.tensor_scalar(
                            rows[:, s:s + 1], idx8[:, s:s + 1],
                            scalar1=T, scalar2=None,
                            op0=mybir.AluOpType.mult)
                        nc.vector.tensor_tensor(
                            out=rows[:, s:s + 1], in0=rows[:, s:s + 1],
                            in1=base[:], op=mybir.AluOpType.add)
                    eo_g = eopool.tile([P, 2, H], f32, tag="eog")
                    for s in range(2):
                        nc.gpsimd.indirect_dma_start(
                            out=eo_g[:, s, :], out_offset=None, in_=eo,
                            in_offset=bass.IndirectOffsetOnAxis(
                                ap=rows[:, s:s + 1], axis=0))
                    g0 = smalls.tile([P, 1], f32, tag="g0")
                    nc.vector.tensor_mul(g0[:], mx8[:, 0:1], rs[:])
                    g1 = smalls.tile([P, 1], f32, tag="g1")
                    nc.vector.tensor_mul(g1[:], mx8[:, 1:2], rs[:])
                    acc = accpool.tile([P, H], f32, tag="acc")
                    nc.vector.tensor_scalar_mul(acc[:], eo_g[:, 0, :], g0[:])
                    nc.vector.scalar_tensor_tensor(
                        out=acc[:], in0=eo_g[:, 1, :], scalar=g1[:], in1=acc[:],
                        op0=mybir.AluOpType.mult, op1=mybir.AluOpType.add)
                    nc.sync.dma_start(out=out[tch * P:(tch + 1) * P, :],
                                      in_=acc[:])

    nc.compile()
    return nc


def _get_nc():
    global _compiled_nc
    if _compiled_nc is None:
        _compiled_nc = _build()
    return _compiled_nc


def kernel(hidden_states, expert_outputs, W1, b1, W2, b2, k=2):
    from concourse.bass_utils import run_bass_kernel_spmd

    hs = np.ascontiguousarray(np.asarray(hidden_states, dtype=np.float32)).reshape(B * S, H)
    eo = np.ascontiguousarray(np.asarray(expert_outputs, dtype=np.float32)).reshape(E, B * S, H)
    w1f = np.asarray(W1, dtype=np.float32)
    w1h = np.ascontiguousarray(w1f.astype(np.float16))
    w1l = np.ascontiguousarray(
        ((w1f.astype(np.float64) - w1h.astype(np.float64)) * 8192.0)
        .astype(np.float16))
    b1v = np.ascontiguousarray(np.asarray(b1, dtype=np.float32))
    w2 = np.ascontiguousarray(np.asarray(W2, dtype=np.float32))
    b2v = np.ascontiguousarray(np.asarray(b2, dtype=np.float32))

    in_maps = []
    for c in range(N_CORES):
        sl = slice(c * T, (c + 1) * T)
        in_maps.append({
            "x": np.ascontiguousarray(hs[sl]),
            "eo": np.ascontiguousarray(eo[:, sl, :]).reshape(E * T, H),
            "w1h": w1h, "w1l": w1l, "b1": b1v, "w2": w2, "b2": b2v,
        })

    nc = _get_nc()
    res = run_bass_kernel_spmd(nc, in_maps, core_ids=list(range(N_CORES)))
    full = np.concatenate([res.results[c]["out"] for c in range(N_CORES)], axis=0)
    return full.reshape(B, S, H)


# revision 11
# speedup vs baseline: 2.0689x; 1.0453x over previous
"""MoE expert-gating kernel for 8 Trainium2 NeuronCores.

Problem (nn_ExpertGating): router MLP (H->H relu, H->E) + softmax + top-2
gating + weighted combine of per-expert outputs.

Sharding: data-parallel over the B*S=8192 tokens -> 1024 tokens per core.
Each core runs the full router for its tokens and combines its slice of all
8 experts' outputs.  No collectives needed; host concatenates the slices.

Per-core pipeline (T=1024 tokens, H=1024, E=8), fp32 throughout (top-2
selection needs fp32 logits: min top2/3 margin on this data is ~5e-6):
  1. transpose x via PE: xT[h, t]
  2. hT = relu(W1.T @ xT + b1)          (PE fp32 2-pass + ACT)
  3. logitsT[e, t] = W2.T @ hT + b2     (PE, W2 stationary -> tiny LDW;
     b2 folded into the PSUM->SBUF copy where experts sit on partitions)
  4. transpose logits chunks back to [t, E], softmax over E, top-2 via
     max8 + match_replace -> dense gates [t, E]
  5. out[t] = sum_e gates[t, e] * eo[e, t]   (DVE scalar_tensor_tensor)

Stage 2-4 run per 512-token half so the first half's combines overlap the
second half's router matmuls.  eo loads stream on the gpsimd (SWDGE) ring
from t=0; x/W/out live on the SP (HWDGE) ring.
"""

import numpy as np

B, S, H, E = 4, 2048, 1024, 8
N_CORES = 8
T = (B * S) // N_CORES  # tokens per core
P = 128  # partitions
TCH = T // P  # token chunks per core (8)
KT = H // P  # contraction tiles (8)
NH = 2  # halves of the token range for stage 2/3
HAL = T // NH  # 512
EH = E // 2  # experts per eo tile (SBUF budget)

_compiled_nc = None


def _build():
    import concourse.bacc as bacc
    import concourse.bass as bass
    import concourse.tile as tile
    from concourse import mybir
    from concourse.masks import make_identity

    f32 = mybir.dt.float32
    nc = bacc.Bacc("TRN2", target_bir_lowering=False, debug=False,
                   num_devices=N_CORES)

    x = nc.dram_tensor("x", [T, H], f32, kind="ExternalInput").ap()
    eo = nc.dram_tensor("eo", [E * T, H], f32, kind="ExternalInput").ap()
    f16 = mybir.dt.float16
    w1h = nc.dram_tensor("w1h", [H, H], f16, kind="ExternalInput").ap()
    w1l = nc.dram_tensor("w1l", [H, H], f16, kind="ExternalInput").ap()
    b1 = nc.dram_tensor("b1", [H], f32, kind="ExternalInput").ap()
    w2 = nc.dram_tensor("w2", [H, E], f32, kind="ExternalInput").ap()
    b2 = nc.dram_tensor("b2", [E], f32, kind="ExternalInput").ap()
    out = nc.dram_tensor("out", [T, H], f32, kind="ExternalOutput").ap()

    with tile.TileContext(nc) as tc:
        with (
            tc.tile_pool(name="singles", bufs=1) as singles,
            tc.tile_pool(name="xpool", bufs=1) as xpool,
            tc.tile_pool(name="eopool", bufs=4) as eopool,
            tc.tile_pool(name="accpool", bufs=3) as accpool,
            tc.tile_pool(name="smalls", bufs=8) as smalls,
            tc.tile_pool(name="ltpool", bufs=2) as ltpool,
            tc.tile_pool(name="tmppool", bufs=3) as tmppool,
            tc.tile_pool(name="hprepool", bufs=2) as hprepool,
            tc.tile_pool(name="psumT", bufs=2, space="PSUM") as psumT,
            tc.tile_pool(name="psum2", bufs=2, space="PSUM") as psum2,
            tc.tile_pool(name="psum2c", bufs=2, space="PSUM") as psum2c,
            tc.tile_pool(name="psum3", bufs=1, space="PSUM") as psum3,
            tc.tile_pool(name="psumL", bufs=1, space="PSUM") as psumL,
        ):
            # ---- x + weights on the SP/HWDGE ring ----
            # ring order: x half0, W1, w2, b1, b2, x half1 (xpool bufs=1)
            x_half = [None, None]
            x_half[0] = xpool.tile([P, TCH // NH, H], f32, tag="x4", name="x4a")
            nc.sync.dma_start(
                out=x_half[0][:, 0:2, :],
                in_=x[0:2 * P, :].rearrange("(a p) h -> p a h", p=P))
            nc.sync.dma_start(
                out=x_half[0][:, 2:4, :],
                in_=x[2 * P:HAL, :].rearrange("(a p) h -> p a h", p=P))
            w1h_sb = singles.tile([P, KT, H], f16)  # fp16 high half of W1
            nc.sync.dma_start(out=w1h_sb[:], in_=w1h.rearrange("(k p) m -> p k m", p=P))
            w1l_sb = singles.tile([P, KT, H], f16)  # fp16 low half of W1, x8192
            nc.sync.dma_start(out=w1l_sb[:], in_=w1l.rearrange("(k p) m -> p k m", p=P))
            w2_sb = singles.tile([P, KT, E], f32)  # w2_sb[p,k,e] = W2[k*128+p, e]
            nc.sync.dma_start(out=w2_sb[:], in_=w2.rearrange("(k p) e -> p k e", p=P))
            b1_sb = singles.tile([P, KT], f32)  # b1_sb[p,m] = b1[m*128+p]
            nc.sync.dma_start(out=b1_sb[:], in_=b1.rearrange("(m p) -> p m", p=P))
            b2_sb = singles.tile([E, 1], f32)  # b2 per partition (expert) for stage3
            nc.sync.dma_start(out=b2_sb[:], in_=b2[:, None])
            x_half[1] = xpool.tile([P, TCH // NH, H], f32, tag="x4", name="x4b")
            nc.sync.dma_start(
                out=x_half[1][:],
                in_=x[HAL:T, :].rearrange("(a p) h -> p a h", p=P))
            ident = singles.tile([P, P], f32)
            make_identity(nc, ident[:])
            iota_u = singles.tile([P, 1], mybir.dt.uint32)
            nc.gpsimd.iota(iota_u[:], pattern=[[0, 1]], base=0,
                           channel_multiplier=1)

            xThi = singles.tile([P, KT, T], f16)  # fp16 high half of x^T
            xTlo = singles.tile([P, KT, T], f16)  # fp16 low half of x^T, x8192
            hT = singles.tile([P, KT, T], f32)  # hT[p,m,t] = relu(x@W1+b1)[t, m*128+p]

            # ---- transposes (PE) + fp16 hi/lo split of x^T ----
            for tch in range(TCH):
                x4 = x_half[tch // (TCH // NH)]
                a = tch % (TCH // NH)
                csl = slice(tch * P, (tch + 1) * P)
                for kk in range(KT // 4):
                    pt = psumT.tile([P, 4 * P], f32, tag="pt")
                    for j in range(4):
                        k = kk * 4 + j
                        nc.tensor.transpose(pt[:, j * P:(j + 1) * P],
                                            x4[:, a, k * P:(k + 1) * P],
                                            ident[:])
                    ksl = slice(kk * 4, (kk + 1) * 4)
                    pt3 = pt[:].rearrange("p (j c) -> p j c", j=4)
                    nc.scalar.copy(out=xThi[:, ksl, csl], in_=pt3)
                    tmp = tmppool.tile([P, 4, P], f32, tag="tmp")
                    nc.vector.tensor_tensor(out=tmp[:], in0=pt3,
                                            in1=xThi[:, ksl, csl],
                                            op=mybir.AluOpType.subtract)
                    nc.vector.tensor_scalar_mul(xTlo[:, ksl, csl], tmp[:], 8192.0)

            SEGS = [(0, 4), (4, 7), (7, 8)]
            for c0, c1 in SEGS:
                sl = slice(c0 * P, c1 * P)
                W = (c1 - c0) * P
                # ---- stage 2: hT = relu(W1.T @ xT + b1), fp16 x3 passes:
                # main = xhi*whi ; corr = (xhi*wlo' + xlo'*whi) with both lo
                # terms pre-scaled by 2^13 -> hT = relu(main + corr/2^13 + b1)
                for m in range(KT):
                    msl = slice(m * P, (m + 1) * P)
                    ps = psum2.tile([P, W], f32, tag="ps", padded_shape=[P, HAL])
                    for k in range(KT):
                        nc.tensor.matmul(
                            ps[:], lhsT=w1h_sb[:, k, msl], rhs=xThi[:, k, sl],
                            start=(k == 0), stop=(k == KT - 1),
                        )
                    psc = psum2c.tile([P, W], f32, tag="psc", padded_shape=[P, HAL])
                    for k in range(KT):
                        nc.tensor.matmul(
                            psc[:], lhsT=w1l_sb[:, k, msl], rhs=xThi[:, k, sl],
                            start=(k == 0), stop=False,
                        )
                        nc.tensor.matmul(
                            psc[:], lhsT=w1h_sb[:, k, msl], rhs=xTlo[:, k, sl],
                            start=False, stop=(k == KT - 1),
                        )
                    hcorr = hprepool.tile([P, W], f32, tag="hcorr", padded_shape=[P, HAL])
                    nc.scalar.mul(out=hcorr[:], in_=psc[:], mul=1.0 / 8192.0)
                    hpre = hprepool.tile([P, W], f32, tag="hpre", padded_shape=[P, HAL])
                    nc.vector.tensor_tensor(out=hpre[:], in0=ps[:], in1=hcorr[:],
                                            op=mybir.AluOpType.add)
                    nc.scalar.activation(
                        out=hT[:, m, sl], in_=hpre[:],
                        func=mybir.ActivationFunctionType.Relu,
                        bias=b1_sb[:, m:m + 1], scale=1.0,
                    )

                # ---- stage 3: logitsT[e, half] = W2.T @ hT (+ b2) ----
                ps3 = psum3.tile([E, W], f32, tag="ps3", padded_shape=[E, HAL])
                for k in range(KT):
                    nc.tensor.matmul(
                        ps3[:],
                        lhsT=w2_sb[:, k, :],
                        rhs=hT[:, k, sl],
                        start=(k == 0), stop=(k == KT - 1),
                    )
                lT = ltpool.tile([E, W], f32, tag="lT", padded_shape=[E, HAL])
                nc.scalar.activation(out=lT[:], in_=ps3[:],
                                     func=mybir.ActivationFunctionType.Identity,
                                     bias=b2_sb[:, 0:1], scale=1.0)

                # ---- stage 4+5 per 128-token chunk: softmax, top-2,
                # indirect gather of the 2 selected expert rows, combine ----
                for tch in range(c0, c1):
                    a = tch - c0
                    pl = psumL.tile([P, E], f32, tag="pl", name="pl")
                    nc.tensor.transpose(pl[:], lT[:, a * P:(a + 1) * P],
                                        ident[:E, :E])
                    negmax = smalls.tile([P, 1], f32, tag="negmax")
                    nc.vector.reduce_max(negmax[:], pl[:],
                                         axis=mybir.AxisListType.X, negate=True)
                    exps = smalls.tile([P, E], f32, tag="exps")
                    nc.scalar.activation(exps[:], pl[:],
                                         func=mybir.ActivationFunctionType.Exp,
                                         bias=negmax[:], scale=1.0)
                    ssum = smalls.tile([P, 1], f32, tag="ssum")
                    nc.vector.reduce_sum(ssum[:], exps[:],
                                         axis=mybir.AxisListType.X)
                    rs = smalls.tile([P, 1], f32, tag="rs")
                    nc.vector.reciprocal(rs[:], ssum[:])
                    # top-2 of exps == top-2 of probs (positive scale);
                    # gate value = exp * (1/sum)
                    mx8 = smalls.tile([P, 8], f32, tag="mx8")
                    nc.vector.max(mx8[:], exps[:])
                    idx8 = smalls.tile([P, 8], mybir.dt.uint32, tag="idx8")
                    nc.vector.max_index(idx8[:], mx8[:], exps[:])
                    # flat eo row = expert*T + (tch*128 + partition)
                    base = smalls.tile([P, 1], mybir.dt.uint32, tag="base")
                    nc.vector.tensor_scalar_add(base[:], iota_u[:], tch * P)
                    rows = smalls.tile([P, 2], mybir.dt.uint32, tag="rows")
                    for s in range(2):
                        nc.vector.tensor_scalar(
                            rows[:, s:s + 1], idx8[:, s:s + 1],
                            scalar1=T, scalar2=None,
                            op0=mybir.AluOpType.mult)
                        nc.vector.tensor_tensor(
                            out=rows[:, s:s + 1], in0=rows[:, s:s + 1],
                            in1=base[:], op=mybir.AluOpType.add)
                    eo_g = eopool.tile([P, 2, H], f32, tag="eog")
                    for s in range(2):
                        nc.gpsimd.indirect_dma_start(
                            out=eo_g[:, s, :], out_offset=None, in_=eo,
                            in_offset=bass.IndirectOffsetOnAxis(
                                ap=rows[:, s:s + 1], axis=0))
                    g0 = smalls.tile([P, 1], f32, tag="g0")
                    nc.vector.tensor_mul(g0[:], mx8[:, 0:1], rs[:])
                    g1 = smalls.tile([P, 1], f32, tag="g1")
                    nc.vector.tensor_mul(g1[:], mx8[:, 1:2], rs[:])
                    acc = accpool.tile([P, H], f32, tag="acc")
                    nc.scalar.activation(acc[:], eo_g[:, 0, :],
                                         func=mybir.ActivationFunctionType.Copy,
                                         scale=g0[:])
                    nc.vector.scalar_tensor_tensor(
                        out=acc[:], in0=eo_g[:, 1, :], scalar=g1[:], in1=acc[:],
                        op0=mybir.AluOpType.mult, op1=mybir.AluOpType.add)
                    nc.sync.dma_start(out=out[tch * P:(tch + 1) * P, :],
                                      in_=acc[:])

    nc.compile()
    return nc


def _get_nc():
    global _compiled_nc
    if _compiled_nc is None:
        _compiled_nc = _build()
    return _compiled_nc


def kernel(hidden_states, expert_outputs, W1, b1, W2, b2, k=2):
    from concourse.bass_utils import run_bass_kernel_spmd

    hs = np.ascontiguousarray(np.asarray(hidden_states, dtype=np.float32)).reshape(B * S, H)
    eo = np.ascontiguousarray(np.asarray(expert_outputs, dtype=np.float32)).reshape(E, B * S, H)
    w1f = np.asarray(W1, dtype=np.float32)
    w1h = np.ascontiguousarray(w1f.astype(np.float16))
    w1l = np.ascontiguousarray(
        ((w1f.astype(np.float64) - w1h.astype(np.float64)) * 8192.0)
        .astype(np.float16))
    b1v = np.ascontiguousarray(np.asarray(b1, dtype=np.float32))
    w2 = np.ascontiguousarray(np.asarray(W2, dtype=np.float32))
    b2v = np.ascontiguousarray(np.asarray(b2, dtype=np.float32))

    in_maps = []
    for c in range(N_CORES):
        sl = slice(c * T, (c + 1) * T)
        in_maps.append({
            "x": np.ascontiguousarray(hs[sl]),
            "eo": np.ascontiguousarray(eo[:, sl, :]).reshape(E * T, H),
            "w1h": w1h, "w1l": w1l, "b1": b1v, "w2": w2, "b2": b2v,
        })

    nc = _get_nc()
    res = run_bass_kernel_spmd(nc, in_maps, core_ids=list(range(N_CORES)))
    full = np.concatenate([res.results[c]["out"] for c in range(N_CORES)], axis=0)
    return full.reshape(B, S, H)
